# revision 16
# baseline (speedup 1.0000x reference)
"""Distance-correlation loss kernel for trn2 (8 NeuronCores, SPMD).

Reference math: for F in {X, Y}: a = sqrt(relu(sq_i + sq_j - 2 F F^T) + eps),
A = a - 2*row_j + tot (row = colsum/(n-2), tot = sum/((n-1)(n-2))), zero diag;
loss = -g_xy / sqrt(g_xx * g_yy + eps), g_PQ = sum(P*Q)/(n(n-3)).

Matrix-free single-pass formulation: with a' = a - 64, every bracket sum
expands as P' (= sum a'_x a'_y and squares) plus O(n) corrections from the
shifted colsums and the measured diagonal — the device computes, per tile,
only the distance tile, its shifted colsum, and three product partials. No
second pass, no collective; the host combines per-core partials in f64.

Symmetric schedule (a is symmetric -> compute ~half): 16 virtual half-blocks
of 256 rows, 2 per core. Core c streams 20 rotated j-chunks (global chunk
(4c+pos)%32); per position the tile is
  pos 0,1:   [128,256] left  (v0 self, weight 1)
  pos 2,3:   [128,512] left w2 + right v1-self w1
  pos 4..15: [128,512] both halves w2
  pos 16,17: [128,512] left w1 (d=8 pair, both orientations), right w2
  pos 18,19: [128,256] right w1 (odd d=8 pair)
Weight-2 halves get transposed-side column sums via ones^T @ a16 matmuls
accumulated in one shared PSUM bank (groups at partitions 0/32/64/96),
emitted with a lag so PE never waits on the ACT/GPSIMD chain. Weights are
baked into the DVE tensor_tensor_reduce `scale`.

Per tile: fp8(e4m3) DoubleRow matmuls (psum = x8_strip^T (-2 x8_core)) plus
one DoubleRow pair encoding sq_i - 2048; ACT sqrt with per-partition bias
sq_j + 2048 + 0.5 (+0.5 keeps the junk diagonal positive -> no relu/NaN);
GPSIMD tensor_scalar shift a-64 -> f16 with accum_out = shifted colsum; DVE
TTR partials. Diagonal blocks sit at stream positions 0..3; their a32 column
blocks are DMA'd out and the host subtracts the exact measured diagonal.

Pipelining: host pre-arranges every fp8 array in exact SBUF layout (fully
contiguous per partition -> no sub-512B DMA penalty); strips are half-width
(1.6us each) for fine overlap; x-strips ride SP, residents the scalar queue,
small tables + y-strips gpsimd; the x stream runs SKEW positions ahead of y
so the PE starts as soon as the first x strip lands.
"""

import sys

for _p in ("/opt/trn_rl_repo",):
    if _p not in sys.path:
        sys.path.insert(0, _p)

import numpy as np
import ml_dtypes

import concourse.bass as bass
from concourse import bacc
import concourse.mybir as mybir
import concourse.tile as tile
from concourse.bass_utils import run_bass_kernel_spmd

N = 4096
D = 2048
NCORES = 8
ROWS = N // NCORES          # 512 resident rows per core
NJ = N // 128               # 32 global j-chunks
NPOS = 20                   # streamed chunk positions per core
NKP = D // 256              # 8 DoubleRow contraction pairs
SKEW = 3                    # x stream runs this many positions ahead of y
K64 = 64.0
EB = 0.5
EPS = 1e-18
F32 = mybir.dt.float32
F16 = mybir.dt.float16
F8 = mybir.dt.float8e4
AF = mybir.ActivationFunctionType
ALU = mybir.AluOpType
DR = mybir.MatmulPerfMode.DoubleRow
f8 = ml_dtypes.float8_e4m3

_CACHE = {}


def _pos_tile(pos):
    """(tile_col_start, tile_width) within the core's 512 resident columns."""
    if pos < 2:
        return 0, 256
    if pos >= 18:
        return 256, 256
    return 0, 512


def _pos_ttrs(pos):
    """(col_start, width, weight) product segments for this position."""
    if pos < 2:
        return [(0, 256, 1.0)]
    if pos < 4:
        return [(0, 256, 2.0), (256, 256, 1.0)]
    if pos < 16:
        return [(0, 512, 2.0)]
    if pos < 18:
        return [(0, 256, 1.0), (256, 256, 2.0)]
    return [(256, 256, 1.0)]


def _pos_mirrors(pos):
    """Mirror groups fed at this position: list of (group, col_start)."""
    out = []
    if 2 <= pos <= 15:
        out.append((0, 0))
    if 4 <= pos <= 17:
        out.append((1, 256))
    return out


NACC = sum(len(_pos_ttrs(p)) for p in range(NPOS))   # accum columns per product
MIR_TOTAL = [14, 14]


def _build_nc():
    nc = bacc.Bacc(None, num_devices=NCORES, target_bir_lowering=False)

    # ---- inputs (pre-arranged in SBUF layout: [128, contiguous bytes]) ----
    # stream: 10 half-strips x [16 kchunks, 256 cols]
    xs8 = nc.declare_dram_parameter("xs8", [128, NPOS * 16 * 128], F8, isOutput=False)
    ys8 = nc.declare_dram_parameter("ys8", [128, NPOS * 16 * 128], F8, isOutput=False)
    # resident moving side (-2 x8): [16 kchunks, 512 cols]
    xm8 = nc.declare_dram_parameter("xm8", [128, 16 * ROWS], F8, isOutput=False)
    ym8 = nc.declare_dram_parameter("ym8", [128, 16 * ROWS], F8, isOutput=False)
    uabx = nc.declare_dram_parameter("uabx", [1, 2 * ROWS], F8, isOutput=False)
    uaby = nc.declare_dram_parameter("uaby", [1, 2 * ROWS], F8, isOutput=False)
    stat = nc.declare_dram_parameter("stat", [1, 256], F8, isOutput=False)
    onesf = nc.declare_dram_parameter("onesf", [128, 1], F16, isOutput=False)
    sqbx = nc.declare_dram_parameter("sqbx", [128, NPOS], F32, isOutput=False)
    sqby = nc.declare_dram_parameter("sqby", [128, NPOS], F32, isOutput=False)

    # ---- outputs ----
    csx = nc.declare_dram_parameter("csx", [128, NPOS], F32, isOutput=True)
    csy = nc.declare_dram_parameter("csy", [128, NPOS], F32, isOutput=True)
    accs = nc.declare_dram_parameter("accs", [128, 3 * NACC], F32, isOutput=True)
    adx = nc.declare_dram_parameter("adx", [128, 512], F32, isOutput=True)
    ady = nc.declare_dram_parameter("ady", [128, 512], F32, isOutput=True)
    mirs = nc.declare_dram_parameter("mirs", [128, 8], F32, isOutput=True)

    with tile.TileContext(nc) as tc:
        import contextlib

        with contextlib.ExitStack() as ctx:
            singles = ctx.enter_context(tc.tile_pool(name="singles", bufs=1))
            xstrips = ctx.enter_context(tc.tile_pool(name="xstrips", bufs=4))
            ystrips = ctx.enter_context(tc.tile_pool(name="ystrips", bufs=4))
            psum = ctx.enter_context(tc.tile_pool(name="psum", bufs=6, space="PSUM"))
            mpsum = ctx.enter_context(tc.tile_pool(name="mpsum", bufs=1, space="PSUM"))
            t32 = ctx.enter_context(tc.tile_pool(name="t32", bufs=6))
            t16 = ctx.enter_context(tc.tile_pool(name="t16", bufs=10))
            scrap = ctx.enter_context(tc.tile_pool(name="scrap", bufs=3))

            # ---- residents ----
            # big moving residents on the scalar (ACT) hwdge queue, split
            # into k-halves so the first matmuls start sooner
            xm_sb = singles.tile([128, 16, ROWS], F8, name="xm_sb")
            nc.scalar.dma_start(out=xm_sb[:, 0:8, :], in_=xm8[:, : 8 * ROWS])
            nc.scalar.dma_start(out=xm_sb[:, 8:16, :], in_=xm8[:, 8 * ROWS :])
            ym_sb = singles.tile([128, 16, ROWS], F8, name="ym_sb")
            nc.scalar.dma_start(out=ym_sb[:, 0:8, :], in_=ym8[:, : 8 * ROWS])
            nc.scalar.dma_start(out=ym_sb[:, 8:16, :], in_=ym8[:, 8 * ROWS :])
            # small tables first on the gpsimd queue
            uabx_sb = singles.tile([1, 2, ROWS], F8, name="uabx_sb")
            nc.gpsimd.dma_start(out=uabx_sb[:], in_=uabx[:, :])
            stat_sb = singles.tile([1, 2, 128], F8, name="stat_sb")
            nc.gpsimd.dma_start(out=stat_sb[:], in_=stat[:, :])
            sqbx_sb = singles.tile([128, NPOS], F32, name="sqbx_sb")
            nc.gpsimd.dma_start(out=sqbx_sb[:], in_=sqbx[:, :])
            uaby_sb = singles.tile([1, 2, ROWS], F8, name="uaby_sb")
            nc.gpsimd.dma_start(out=uaby_sb[:], in_=uaby[:, :])
            sqby_sb = singles.tile([128, NPOS], F32, name="sqby_sb")
            nc.gpsimd.dma_start(out=sqby_sb[:], in_=sqby[:, :])
            ones_sb = singles.tile([128, 1], F16, name="ones_sb")
            nc.gpsimd.dma_start(out=ones_sb[:], in_=onesf[:, :])

            csx_sb = singles.tile([128, NPOS], F32, name="csx_sb")
            csy_sb = singles.tile([128, NPOS], F32, name="csy_sb")
            accs_sb = singles.tile([128, 3 * NACC], F32, name="accs_sb")
            mir_sb = singles.tile([128, 8], F32, name="mir_sb")

            # Mirror sums via the stationary trick: lhsT = a16 column chunk,
            # rhs = ones [128,1] -> out[c,0] = sum_p a16[p,c]. Output free
            # size 1 makes these matmuls ~free on the PE. All 8 accumulator
            # columns (m,g,chunk) share one PSUM bank: the first matmul's
            # start=True zeroes the whole 2KB region (emission order on the
            # in-order PE guarantees it runs first), the very last carries
            # stop=True.
            mir_ps = mpsum.tile([128, 8], F32, name="mir_ps")
            mir_emitted = [0]
            MIR_MM_TOTAL = 2 * 2 * MIR_TOTAL[0] * 2   # m * chunks * positions
            pending_mirrors = []

            def flush_mirror():
                m, g, a16t, rel = pending_mirrors.pop(0)
                for chunk in range(2):
                    col = 4 * m + 2 * g + chunk
                    mir_emitted[0] += 1
                    nc.tensor.matmul(
                        mir_ps[:, col : col + 1],
                        lhsT=a16t[:, rel + 128 * chunk : rel + 128 * chunk + 128],
                        rhs=ones_sb[:],
                        start=(mir_emitted[0] == 1),
                        stop=(mir_emitted[0] == MIR_MM_TOTAL),
                    )
                if mir_emitted[0] == MIR_MM_TOTAL:
                    nc.scalar.activation(
                        mir_sb[:], mir_ps[:],
                        AF.Copy, bias=0.0, scale=1.0,
                    )

            strips = [[None] * (NPOS // 2) for _ in range(2)]

            def load_strip(m, h):
                pool, eng = (xstrips, nc.sync) if m == 0 else (ystrips, nc.gpsimd)
                src = xs8 if m == 0 else ys8
                st = pool.tile([128, 16, 256], F8, tag="st")
                eng.dma_start(out=st[:], in_=src[:, 4096 * h : 4096 * (h + 1)])
                strips[m][h] = st

            sides = (
                (xm_sb, uabx_sb, sqbx_sb, csx_sb, adx),
                (ym_sb, uaby_sb, sqby_sb, csy_sb, ady),
            )
            a16_live = [{}, {}]
            acc_col_of = {}
            _c = 0
            for pos in range(NPOS):
                acc_col_of[pos] = _c
                _c += len(_pos_ttrs(pos))

            def emit_tile(m, pos):
                m_sb, uab_sb, sqb_sb, cs_sb, ad = sides[m]
                h = pos // 2
                t = pos % 2
                if strips[m][h] is None:
                    load_strip(m, h)
                    if h + 1 < NPOS // 2 and strips[m][h + 1] is None:
                        load_strip(m, h + 1)
                strip = strips[m][h]
                c0, cw = _pos_tile(pos)
                ps = psum.tile([128, cw], F32, tag="mm")
                for kp in range(NKP):
                    nc.tensor.matmul(
                        ps[:],
                        lhsT=strip[:, 2 * kp : 2 * kp + 2, 128 * t : 128 * t + 128],
                        rhs=m_sb[:, 2 * kp : 2 * kp + 2, c0 : c0 + cw],
                        start=(kp == 0),
                        stop=False,
                        perf_mode=DR,
                    )
                nc.tensor.matmul(
                    ps[:], lhsT=stat_sb[:], rhs=uab_sb[:, :, c0 : c0 + cw],
                    start=False, stop=True, perf_mode=DR,
                )
                a32 = t32.tile([128, cw], F32, tag="a32")
                nc.scalar.activation(
                    a32[:], ps[:], AF.Sqrt,
                    bias=sqb_sb[:, pos : pos + 1], scale=1.0,
                )
                a16 = t16.tile([128, cw], F16, tag="a16")
                # x-shifts ride DVE (slack); y-shifts stay on gpsimd so the
                # TTRs (DVE, in-order) are never blocked behind x work
                eng = nc.vector if m == 0 else nc.gpsimd
                eng.tensor_scalar(
                    a16[:], a32[:], -K64, None,
                    op0=ALU.add, op1=ALU.add,
                    accum_out=cs_sb[:, pos : pos + 1],
                )
                a16_live[m][pos] = a16
                if pos < 4:
                    nc.sync.dma_start(
                        out=ad[:, 128 * pos : 128 * pos + 128],
                        in_=a32[:, 128 * pos - c0 : 128 * pos - c0 + 128],
                    )
                for g, gcol in _pos_mirrors(pos):
                    pending_mirrors.append((m, g, a16, gcol - c0))

            def emit_ttrs(pos):
                c0, _ = _pos_tile(pos)
                a16x = a16_live[0].pop(pos)
                a16y = a16_live[1][pos]
                acc_col = acc_col_of[pos]
                for seg0, segw, segwt in _pos_ttrs(pos):
                    for k, (i0t, i1t) in enumerate(
                        ((a16x, a16y), (a16x, a16x), (a16y, a16y))
                    ):
                        scr = scrap.tile([128, segw], F16, tag="scr")
                        nc.vector.tensor_tensor_reduce(
                            out=scr[:],
                            in0=i0t[:, seg0 - c0 : seg0 - c0 + segw],
                            in1=i1t[:, seg0 - c0 : seg0 - c0 + segw],
                            scale=segwt, scalar=0.0,
                            op0=ALU.mult, op1=ALU.add,
                            accum_out=accs_sb[:, k * NACC + acc_col : k * NACC + acc_col + 1],
                        )
                    acc_col += 1
                a16_live[1].pop(pos)

            # heavy [512] positions first; light [256] self/d8 tiles last so
            # the drain chain is short
            ORDER = list(range(2, 18)) + [0, 1, 18, 19]
            # prime the first two distinct x strips
            primed = []
            for o in ORDER:
                if o // 2 not in primed:
                    primed.append(o // 2)
                if len(primed) == 2:
                    break
            for h in primed:
                load_strip(0, h)
            for i in range(NPOS + SKEW):
                if i >= SKEW:
                    pos = ORDER[i - SKEW]
                    emit_tile(1, pos)
                    emit_ttrs(pos)
                if i < NPOS:
                    emit_tile(0, ORDER[i])
                while len(pending_mirrors) > 6:
                    flush_mirror()
            while pending_mirrors:
                flush_mirror()

            nc.sync.dma_start(out=csx[:, :], in_=csx_sb[:])
            nc.sync.dma_start(out=csy[:, :], in_=csy_sb[:])
            nc.sync.dma_start(out=accs[:, :], in_=accs_sb[:])
            nc.sync.dma_start(out=mirs[:, :], in_=mir_sb[:])

    nc.compile()
    return nc


def _get_nc():
    if "nc" not in _CACHE:
        _CACHE["nc"] = _build_nc()
    return _CACHE["nc"]


def _prep_side(F):
    x8 = np.asarray(F, dtype=np.float32).reshape(N, D).astype(f8).astype(np.float32)
    xsT = np.ascontiguousarray(x8.T).astype(f8)                 # [D, N]
    xmT = np.ascontiguousarray((-2.0 * x8).T).astype(f8)        # [D, N]
    sq = np.einsum("ij,ij->i", x8.astype(np.float64), x8.astype(np.float64))
    u = sq - 2048.0
    uA = (u / 16.0).astype(f8)
    uB = ((u - uA.astype(np.float64) * 16.0) / 2.0).astype(f8)
    sqb = (sq + 2048.0 + EB).astype(np.float32)
    return xsT, xmT, np.asarray(uA), np.asarray(uB), sqb


def _sbuf_arrange_stream(arr, c):
    """[D, N] -> [128, NPOS*16*128]: half-strip h holds k-chunks of rotated
    columns [128h, 128h+128) x [128 cols] in [k][col] order per partition."""
    start = 512 * c
    end = start + NPOS * 128
    if end <= N:
        w = arr[:, start:end]
    else:
        w = np.concatenate([arr[:, start:], arr[:, : end - N]], axis=1)
    # w: [D, NPOS*128]; per partition p: [halfstrip][k][col256], D = (k p)
    v = w.reshape(16, 128, NPOS // 2, 256)       # [k, p, hs, col]
    v = v.transpose(1, 2, 0, 3)                  # [p, hs, k, col]
    return np.ascontiguousarray(v.reshape(128, NPOS * 16 * 128))


def _sbuf_arrange_resident(arr_sl):
    """[D, ROWS] -> [128, 16*ROWS] in [k][col] order per partition."""
    v = arr_sl.reshape(16, 128, ROWS)            # [k, p, col]
    v = v.transpose(1, 0, 2)                     # [p, k, col]
    return np.ascontiguousarray(v.reshape(128, 16 * ROWS))


def _make_in_maps(featuresX, featuresY):
    xsT, xmT, uAx, uBx, sqbx = _prep_side(featuresX)
    ysT, ymT, uAy, uBy, sqby = _prep_side(featuresY)
    stat_np = np.concatenate(
        [np.full(128, 16.0, np.float32), np.full(128, 2.0, np.float32)]
    ).astype(f8).reshape(1, 256)
    ones_np = np.ones((128, 1), np.float16)

    in_maps = []
    for c in range(NCORES):
        sl = slice(c * ROWS, (c + 1) * ROWS)
        rot = [(4 * c + pos) % NJ for pos in range(NPOS)]
        sqbx_c = np.stack([sqbx[128 * g : 128 * g + 128] for g in rot], axis=1)
        sqby_c = np.stack([sqby[128 * g : 128 * g + 128] for g in rot], axis=1)
        in_maps.append(
            {
                "xs8": _sbuf_arrange_stream(xsT, c),
                "ys8": _sbuf_arrange_stream(ysT, c),
                "xm8": _sbuf_arrange_resident(xmT[:, sl]),
                "ym8": _sbuf_arrange_resident(ymT[:, sl]),
                "uabx": np.concatenate([uAx[sl], uBx[sl]]).reshape(1, 2 * ROWS),
                "uaby": np.concatenate([uAy[sl], uBy[sl]]).reshape(1, 2 * ROWS),
                "stat": stat_np,
                "onesf": ones_np,
                "sqbx": np.ascontiguousarray(sqbx_c),
                "sqby": np.ascontiguousarray(sqby_c),
            }
        )
    return in_maps


def _combine(res):
    cspx = np.zeros(N, np.float64)
    cspy = np.zeros(N, np.float64)
    P = np.zeros(3, np.float64)
    adiag_x = np.zeros(N, np.float64)
    adiag_y = np.zeros(N, np.float64)
    for c in range(NCORES):
        r = res[c]
        for pos in range(NPOS):
            gj = (4 * c + pos) % NJ
            cspx[128 * gj : 128 * gj + 128] += r["csx"][:, pos].astype(np.float64)
            cspy[128 * gj : 128 * gj + 128] += r["csy"][:, pos].astype(np.float64)
        P += r["accs"].astype(np.float64).reshape(128, 3, NACC).sum(axis=(0, 2))
        i0 = 512 * c
        mir = r["mirs"].astype(np.float64)   # [p, 4m+2g+chunk]
        for m, csp in ((0, cspx), (1, cspy)):
            for g in range(2):
                for chunk in range(2):
                    b = i0 + 256 * g + 128 * chunk
                    csp[b : b + 128] += mir[:, 4 * m + 2 * g + chunk]
        for t in range(4):
            blk_x = r["adx"][:, 128 * t : 128 * t + 128]
            blk_y = r["ady"][:, 128 * t : 128 * t + 128]
            adiag_x[i0 + 128 * t : i0 + 128 * t + 128] = np.diagonal(blk_x).astype(np.float64)
            adiag_y[i0 + 128 * t : i0 + 128 * t + 128] = np.diagonal(blk_y).astype(np.float64)

    def bracket(Pv, c1p, c2p, d1, d2_):
        n = float(N)
        r1 = c1p / (n - 2)
        r2 = c2p / (n - 2)
        t1 = c1p.sum() / ((n - 1) * (n - 2)) - K64 / (n - 1)
        t2 = c2p.sum() / ((n - 1) * (n - 2)) - K64 / (n - 1)
        sv = Pv
        sv += -2.0 * (r2 @ c1p) + t2 * c1p.sum()
        sv += -2.0 * (r1 @ c2p) + t1 * c2p.sum()
        sv += 4.0 * n * (r1 @ r2)
        sv += -2.0 * n * t2 * r1.sum() - 2.0 * n * t1 * r2.sum()
        sv += n * n * t1 * t2
        A_ii = (d1 - K64) - 2.0 * r1 + t1
        B_ii = (d2_ - K64) - 2.0 * r2 + t2
        sv -= (A_ii * B_ii).sum()
        return sv / (n * (n - 3.0))

    gxy = bracket(P[0], cspx, cspy, adiag_x, adiag_y)
    gxx = bracket(P[1], cspx, cspx, adiag_x, adiag_x)
    gyy = bracket(P[2], cspy, cspy, adiag_y, adiag_y)
    loss = -gxy / np.sqrt(gxx * gyy + EPS)
    return np.array(loss, dtype=np.float32)


def kernel(featuresX: np.ndarray, featuresY: np.ndarray) -> np.ndarray:
    nc = _get_nc()
    in_maps = _make_in_maps(featuresX, featuresY)
    _CACHE["in_maps"] = in_maps
    res = run_bass_kernel_spmd(nc, in_maps, list(range(NCORES))).results
    return _combine(res)


# revision 19
# speedup vs baseline: 1.1036x; 1.1036x over previous
"""Distance-correlation loss kernel for trn2 (8 NeuronCores, SPMD).

Reference math: for F in {X, Y}: a = sqrt(relu(sq_i + sq_j - 2 F F^T) + eps),
A = a - 2*row_j + tot (row = colsum/(n-2), tot = sum/((n-1)(n-2))), zero diag;
loss = -g_xy / sqrt(g_xx * g_yy + eps), g_PQ = sum(P*Q)/(n(n-3)).

Matrix-free single-pass formulation: with a' = a - 64, every bracket sum
expands as P' (= sum a'_x a'_y and squares) plus O(n) corrections from the
shifted colsums and the measured diagonal — the device computes, per tile,
only the distance tile, its shifted colsum, and three product partials. No
second pass, no collective; the host combines per-core partials in f64.

Symmetric schedule (a is symmetric -> compute ~half): 16 virtual half-blocks
of 256 rows, 2 per core. Core c streams 20 rotated j-chunks (global chunk
(4c+pos)%32); per position the tile is
  pos 0,1:   [128,256] left  (v0 self, weight 1)
  pos 2,3:   [128,512] left w2 + right v1-self w1
  pos 4..15: [128,512] both halves w2
  pos 16,17: [128,512] left w1 (d=8 pair, both orientations), right w2
  pos 18,19: [128,256] right w1 (odd d=8 pair)
Weight-2 halves get transposed-side column sums via ones^T @ a16 matmuls
accumulated in one shared PSUM bank (groups at partitions 0/32/64/96),
emitted with a lag so PE never waits on the ACT/GPSIMD chain. Weights are
baked into the DVE tensor_tensor_reduce `scale`.

Per tile: fp8(e4m3) DoubleRow matmuls (psum = x8_strip^T (-2 x8_core)) plus
one DoubleRow pair encoding sq_i - 2048; ACT sqrt with per-partition bias
sq_j + 2048 + 0.5 (+0.5 keeps the junk diagonal positive -> no relu/NaN);
GPSIMD tensor_scalar shift a-64 -> f16 with accum_out = shifted colsum; DVE
TTR partials. Diagonal blocks sit at stream positions 0..3; their a32 column
blocks are DMA'd out and the host subtracts the exact measured diagonal.

Pipelining: host pre-arranges every fp8 array in exact SBUF layout (fully
contiguous per partition -> no sub-512B DMA penalty); strips are half-width
(1.6us each) for fine overlap; x-strips ride SP, residents the scalar queue,
small tables + y-strips gpsimd; the x stream runs SKEW positions ahead of y
so the PE starts as soon as the first x strip lands.
"""

import sys

for _p in ("/opt/trn_rl_repo",):
    if _p not in sys.path:
        sys.path.insert(0, _p)

import numpy as np
import ml_dtypes

import concourse.bass as bass
from concourse import bacc
import concourse.mybir as mybir
import concourse.tile as tile
from concourse.bass_utils import run_bass_kernel_spmd

N = 4096
D = 2048
NCORES = 8
ROWS = N // NCORES          # 512 resident rows per core
NJ = N // 128               # 32 global j-chunks
NPOS = 20                   # streamed chunk positions per core
NKP = D // 256              # 8 DoubleRow contraction pairs
SKEW = 3                    # x stream runs this many positions ahead of y
K64 = 64.0
EB = 0.5
EPS = 1e-18
F32 = mybir.dt.float32
F16 = mybir.dt.float16
F8 = mybir.dt.float8e4
AF = mybir.ActivationFunctionType
ALU = mybir.AluOpType
DR = mybir.MatmulPerfMode.DoubleRow
f8 = ml_dtypes.float8_e4m3

_CACHE = {}


def _pos_tile(pos):
    """(tile_col_start, tile_width) within the core's 512 resident columns."""
    if pos < 2:
        return 0, 256
    if pos >= 18:
        return 256, 256
    return 0, 512


def _pos_ttrs(pos):
    """(col_start, width, weight) product segments for this position."""
    if pos < 2:
        return [(0, 256, 1.0)]
    if pos < 4:
        return [(0, 256, 2.0), (256, 256, 1.0)]
    if pos < 16:
        return [(0, 512, 2.0)]
    if pos < 18:
        return [(0, 256, 1.0), (256, 256, 2.0)]
    return [(256, 256, 1.0)]


def _pos_mirrors(pos):
    """Mirror groups fed at this position: list of (group, col_start)."""
    out = []
    if 2 <= pos <= 15:
        out.append((0, 0))
    if 4 <= pos <= 17:
        out.append((1, 256))
    return out


NACC = sum(len(_pos_ttrs(p)) for p in range(NPOS))   # accum columns per product
MIR_TOTAL = [14, 14]


def _build_nc():
    nc = bacc.Bacc(None, num_devices=NCORES, target_bir_lowering=False)

    # ---- inputs (pre-arranged in SBUF layout: [128, contiguous bytes]) ----
    # stream: 10 half-strips x [16 kchunks, 256 cols]
    xs8 = nc.declare_dram_parameter("xs8", [128, NPOS * 16 * 128], F8, isOutput=False)
    ys8 = nc.declare_dram_parameter("ys8", [128, NPOS * 16 * 128], F8, isOutput=False)
    # resident moving side (-2 x8): [16 kchunks, 512 cols]
    xm8 = nc.declare_dram_parameter("xm8", [128, 16 * ROWS], F8, isOutput=False)
    ym8 = nc.declare_dram_parameter("ym8", [128, 16 * ROWS], F8, isOutput=False)
    uabx = nc.declare_dram_parameter("uabx", [1, 2 * ROWS], F8, isOutput=False)
    uaby = nc.declare_dram_parameter("uaby", [1, 2 * ROWS], F8, isOutput=False)
    stat = nc.declare_dram_parameter("stat", [1, 256], F8, isOutput=False)
    onesf = nc.declare_dram_parameter("onesf", [128, 1], F16, isOutput=False)
    sqbx = nc.declare_dram_parameter("sqbx", [128, NPOS], F32, isOutput=False)
    sqby = nc.declare_dram_parameter("sqby", [128, NPOS], F32, isOutput=False)

    # ---- outputs ----
    csx = nc.declare_dram_parameter("csx", [128, NPOS], F32, isOutput=True)
    csy = nc.declare_dram_parameter("csy", [128, NPOS], F32, isOutput=True)
    accs = nc.declare_dram_parameter("accs", [128, 3 * NACC], F32, isOutput=True)
    adx = nc.declare_dram_parameter("adx", [128, 512], F32, isOutput=True)
    ady = nc.declare_dram_parameter("ady", [128, 512], F32, isOutput=True)
    mirs = nc.declare_dram_parameter("mirs", [128, 8], F32, isOutput=True)

    with tile.TileContext(nc) as tc:
        import contextlib

        with contextlib.ExitStack() as ctx:
            singles = ctx.enter_context(tc.tile_pool(name="singles", bufs=1))
            xstrips = ctx.enter_context(tc.tile_pool(name="xstrips", bufs=4))
            ystrips = ctx.enter_context(tc.tile_pool(name="ystrips", bufs=4))
            psum = ctx.enter_context(tc.tile_pool(name="psum", bufs=7, space="PSUM"))
            mpsum = ctx.enter_context(tc.tile_pool(name="mpsum", bufs=1, space="PSUM"))
            t32 = ctx.enter_context(tc.tile_pool(name="t32", bufs=6))
            t16 = ctx.enter_context(tc.tile_pool(name="t16", bufs=10))
            scrap = ctx.enter_context(tc.tile_pool(name="scrap", bufs=3))

            # ---- residents ----
            # big moving residents on the scalar (ACT) hwdge queue, split
            # into k-halves so the first matmuls start sooner
            xm_sb = singles.tile([128, 16, ROWS], F8, name="xm_sb")
            nc.scalar.dma_start(out=xm_sb[:, 0:8, :], in_=xm8[:, : 8 * ROWS])
            nc.scalar.dma_start(out=xm_sb[:, 8:16, :], in_=xm8[:, 8 * ROWS :])
            ym_sb = singles.tile([128, 16, ROWS], F8, name="ym_sb")
            nc.scalar.dma_start(out=ym_sb[:, 0:8, :], in_=ym8[:, : 8 * ROWS])
            nc.scalar.dma_start(out=ym_sb[:, 8:16, :], in_=ym8[:, 8 * ROWS :])
            # small tables first on the gpsimd queue
            uabx_sb = singles.tile([1, 2, ROWS], F8, name="uabx_sb")
            nc.gpsimd.dma_start(out=uabx_sb[:], in_=uabx[:, :])
            stat_sb = singles.tile([1, 2, 128], F8, name="stat_sb")
            nc.gpsimd.dma_start(out=stat_sb[:], in_=stat[:, :])
            sqbx_sb = singles.tile([128, NPOS], F32, name="sqbx_sb")
            nc.gpsimd.dma_start(out=sqbx_sb[:], in_=sqbx[:, :])
            uaby_sb = singles.tile([1, 2, ROWS], F8, name="uaby_sb")
            nc.gpsimd.dma_start(out=uaby_sb[:], in_=uaby[:, :])
            sqby_sb = singles.tile([128, NPOS], F32, name="sqby_sb")
            nc.gpsimd.dma_start(out=sqby_sb[:], in_=sqby[:, :])
            ones_sb = singles.tile([128, 1], F16, name="ones_sb")
            nc.gpsimd.dma_start(out=ones_sb[:], in_=onesf[:, :])

            csx_sb = singles.tile([128, NPOS], F32, name="csx_sb")
            csy_sb = singles.tile([128, NPOS], F32, name="csy_sb")
            accs_sb = singles.tile([128, 3 * NACC], F32, name="accs_sb")
            mir_sb = singles.tile([128, 8], F32, name="mir_sb")

            # Mirror sums via the stationary trick: lhsT = a16 column chunk,
            # rhs = ones [128,1] -> out[c,0] = sum_p a16[p,c]. Output free
            # size 1 makes these matmuls ~free on the PE. All 8 accumulator
            # columns (m,g,chunk) share one PSUM bank: the first matmul's
            # start=True zeroes the whole 2KB region (emission order on the
            # in-order PE guarantees it runs first), the very last carries
            # stop=True.
            mir_ps = mpsum.tile([128, 8], F32, name="mir_ps")
            mir_emitted = [0]
            MIR_MM_TOTAL = 2 * 2 * MIR_TOTAL[0] * 2   # m * chunks * positions
            pending_mirrors = []

            def flush_mirror():
                m, g, a16t, rel = pending_mirrors.pop(0)
                for chunk in range(2):
                    col = 4 * m + 2 * g + chunk
                    mir_emitted[0] += 1
                    nc.tensor.matmul(
                        mir_ps[:, col : col + 1],
                        lhsT=a16t[:, rel + 128 * chunk : rel + 128 * chunk + 128],
                        rhs=ones_sb[:],
                        start=(mir_emitted[0] == 1),
                        stop=(mir_emitted[0] == MIR_MM_TOTAL),
                    )
                if mir_emitted[0] == MIR_MM_TOTAL:
                    nc.scalar.activation(
                        mir_sb[:], mir_ps[:],
                        AF.Copy, bias=0.0, scale=1.0,
                    )

            strips = [[None] * (NPOS // 2) for _ in range(2)]

            def load_strip(m, h):
                pool, eng = (xstrips, nc.sync) if m == 0 else (ystrips, nc.gpsimd)
                src = xs8 if m == 0 else ys8
                st = pool.tile([128, 16, 256], F8, tag="st")
                eng.dma_start(out=st[:], in_=src[:, 4096 * h : 4096 * (h + 1)])
                strips[m][h] = st

            sides = (
                (xm_sb, uabx_sb, sqbx_sb, csx_sb, adx),
                (ym_sb, uaby_sb, sqby_sb, csy_sb, ady),
            )
            a16_live = [{}, {}]
            acc_col_of = {}
            _c = 0
            for pos in range(NPOS):
                acc_col_of[pos] = _c
                _c += len(_pos_ttrs(pos))

            def emit_tile(m, pos):
                m_sb, uab_sb, sqb_sb, cs_sb, ad = sides[m]
                h = pos // 2
                t = pos % 2
                if strips[m][h] is None:
                    load_strip(m, h)
                    if h + 1 < NPOS // 2 and strips[m][h + 1] is None:
                        load_strip(m, h + 1)
                strip = strips[m][h]
                c0, cw = _pos_tile(pos)
                ps = psum.tile([128, cw], F32, tag="mm")
                for kp in range(NKP):
                    nc.tensor.matmul(
                        ps[:],
                        lhsT=strip[:, 2 * kp : 2 * kp + 2, 128 * t : 128 * t + 128],
                        rhs=m_sb[:, 2 * kp : 2 * kp + 2, c0 : c0 + cw],
                        start=(kp == 0),
                        stop=False,
                        perf_mode=DR,
                    )
                nc.tensor.matmul(
                    ps[:], lhsT=stat_sb[:], rhs=uab_sb[:, :, c0 : c0 + cw],
                    start=False, stop=True, perf_mode=DR,
                )
                a32 = t32.tile([128, cw], F32, tag="a32")
                nc.scalar.activation(
                    a32[:], ps[:], AF.Sqrt,
                    bias=sqb_sb[:, pos : pos + 1], scale=1.0,
                )
                a16 = t16.tile([128, cw], F16, tag="a16")
                nc.gpsimd.tensor_scalar(
                    a16[:], a32[:], -K64, None,
                    op0=ALU.add, op1=ALU.add,
                    accum_out=cs_sb[:, pos : pos + 1],
                )
                a16_live[m][pos] = a16
                if pos < 4:
                    nc.sync.dma_start(
                        out=ad[:, 128 * pos : 128 * pos + 128],
                        in_=a32[:, 128 * pos - c0 : 128 * pos - c0 + 128],
                    )
                for g, gcol in _pos_mirrors(pos):
                    pending_mirrors.append((m, g, a16, gcol - c0))

            def emit_ttrs(pos):
                c0, _ = _pos_tile(pos)
                a16x = a16_live[0].pop(pos)
                a16y = a16_live[1][pos]
                acc_col = acc_col_of[pos]
                for seg0, segw, segwt in _pos_ttrs(pos):
                    for k, (i0t, i1t) in enumerate(
                        ((a16x, a16y), (a16x, a16x), (a16y, a16y))
                    ):
                        scr = scrap.tile([128, segw], F16, tag="scr")
                        nc.vector.tensor_tensor_reduce(
                            out=scr[:],
                            in0=i0t[:, seg0 - c0 : seg0 - c0 + segw],
                            in1=i1t[:, seg0 - c0 : seg0 - c0 + segw],
                            scale=segwt, scalar=0.0,
                            op0=ALU.mult, op1=ALU.add,
                            accum_out=accs_sb[:, k * NACC + acc_col : k * NACC + acc_col + 1],
                        )
                    acc_col += 1
                a16_live[1].pop(pos)

            # heavy [512] positions first; light [256] self/d8 tiles last so
            # the drain chain is short
            ORDER = list(range(2, 18)) + [0, 1, 18, 19]
            # prime the first two distinct x strips
            primed = []
            for o in ORDER:
                if o // 2 not in primed:
                    primed.append(o // 2)
                if len(primed) == 2:
                    break
            for h in primed:
                load_strip(0, h)
            # y starts SKEW positions behind x (residents arrive later) and
            # catches up to 1 behind mid-stream so the drain tail is short
            y_next = 0
            for i in range(NPOS + 1):
                ycount = 0 if i < SKEW else (2 if i in (NPOS // 2, NPOS // 2 + 1) else 1)
                if i == NPOS:
                    ycount = NPOS - y_next
                for _ in range(ycount):
                    if y_next < NPOS and y_next < i:
                        pos = ORDER[y_next]
                        emit_tile(1, pos)
                        emit_ttrs(pos)
                        y_next += 1
                if i < NPOS:
                    emit_tile(0, ORDER[i])
                while len(pending_mirrors) > 6:
                    flush_mirror()
            while y_next < NPOS:
                pos = ORDER[y_next]
                emit_tile(1, pos)
                emit_ttrs(pos)
                y_next += 1
            while pending_mirrors:
                flush_mirror()

            nc.sync.dma_start(out=csx[:, :], in_=csx_sb[:])
            nc.sync.dma_start(out=csy[:, :], in_=csy_sb[:])
            nc.sync.dma_start(out=accs[:, :], in_=accs_sb[:])
            nc.sync.dma_start(out=mirs[:, :], in_=mir_sb[:])

    nc.compile()
    return nc


def _get_nc():
    if "nc" not in _CACHE:
        _CACHE["nc"] = _build_nc()
    return _CACHE["nc"]


def _prep_side(F):
    x8 = np.asarray(F, dtype=np.float32).reshape(N, D).astype(f8).astype(np.float32)
    xsT = np.ascontiguousarray(x8.T).astype(f8)                 # [D, N]
    xmT = np.ascontiguousarray((-2.0 * x8).T).astype(f8)        # [D, N]
    sq = np.einsum("ij,ij->i", x8.astype(np.float64), x8.astype(np.float64))
    u = sq - 2048.0
    uA = (u / 16.0).astype(f8)
    uB = ((u - uA.astype(np.float64) * 16.0) / 2.0).astype(f8)
    sqb = (sq + 2048.0 + EB).astype(np.float32)
    return xsT, xmT, np.asarray(uA), np.asarray(uB), sqb


def _sbuf_arrange_stream(arr, c):
    """[D, N] -> [128, NPOS*16*128]: half-strip h holds k-chunks of rotated
    columns [128h, 128h+128) x [128 cols] in [k][col] order per partition."""
    start = 512 * c
    end = start + NPOS * 128
    if end <= N:
        w = arr[:, start:end]
    else:
        w = np.concatenate([arr[:, start:], arr[:, : end - N]], axis=1)
    # w: [D, NPOS*128]; per partition p: [halfstrip][k][col256], D = (k p)
    v = w.reshape(16, 128, NPOS // 2, 256)       # [k, p, hs, col]
    v = v.transpose(1, 2, 0, 3)                  # [p, hs, k, col]
    return np.ascontiguousarray(v.reshape(128, NPOS * 16 * 128))


def _sbuf_arrange_resident(arr_sl):
    """[D, ROWS] -> [128, 16*ROWS] in [k][col] order per partition."""
    v = arr_sl.reshape(16, 128, ROWS)            # [k, p, col]
    v = v.transpose(1, 0, 2)                     # [p, k, col]
    return np.ascontiguousarray(v.reshape(128, 16 * ROWS))


def _make_in_maps(featuresX, featuresY):
    xsT, xmT, uAx, uBx, sqbx = _prep_side(featuresX)
    ysT, ymT, uAy, uBy, sqby = _prep_side(featuresY)
    stat_np = np.concatenate(
        [np.full(128, 16.0, np.float32), np.full(128, 2.0, np.float32)]
    ).astype(f8).reshape(1, 256)
    ones_np = np.ones((128, 1), np.float16)

    in_maps = []
    for c in range(NCORES):
        sl = slice(c * ROWS, (c + 1) * ROWS)
        rot = [(4 * c + pos) % NJ for pos in range(NPOS)]
        sqbx_c = np.stack([sqbx[128 * g : 128 * g + 128] for g in rot], axis=1)
        sqby_c = np.stack([sqby[128 * g : 128 * g + 128] for g in rot], axis=1)
        in_maps.append(
            {
                "xs8": _sbuf_arrange_stream(xsT, c),
                "ys8": _sbuf_arrange_stream(ysT, c),
                "xm8": _sbuf_arrange_resident(xmT[:, sl]),
                "ym8": _sbuf_arrange_resident(ymT[:, sl]),
                "uabx": np.concatenate([uAx[sl], uBx[sl]]).reshape(1, 2 * ROWS),
                "uaby": np.concatenate([uAy[sl], uBy[sl]]).reshape(1, 2 * ROWS),
                "stat": stat_np,
                "onesf": ones_np,
                "sqbx": np.ascontiguousarray(sqbx_c),
                "sqby": np.ascontiguousarray(sqby_c),
            }
        )
    return in_maps


def _combine(res):
    cspx = np.zeros(N, np.float64)
    cspy = np.zeros(N, np.float64)
    P = np.zeros(3, np.float64)
    adiag_x = np.zeros(N, np.float64)
    adiag_y = np.zeros(N, np.float64)
    for c in range(NCORES):
        r = res[c]
        for pos in range(NPOS):
            gj = (4 * c + pos) % NJ
            cspx[128 * gj : 128 * gj + 128] += r["csx"][:, pos].astype(np.float64)
            cspy[128 * gj : 128 * gj + 128] += r["csy"][:, pos].astype(np.float64)
        P += r["accs"].astype(np.float64).reshape(128, 3, NACC).sum(axis=(0, 2))
        i0 = 512 * c
        mir = r["mirs"].astype(np.float64)   # [p, 4m+2g+chunk]
        for m, csp in ((0, cspx), (1, cspy)):
            for g in range(2):
                for chunk in range(2):
                    b = i0 + 256 * g + 128 * chunk
                    csp[b : b + 128] += mir[:, 4 * m + 2 * g + chunk]
        for t in range(4):
            blk_x = r["adx"][:, 128 * t : 128 * t + 128]
            blk_y = r["ady"][:, 128 * t : 128 * t + 128]
            adiag_x[i0 + 128 * t : i0 + 128 * t + 128] = np.diagonal(blk_x).astype(np.float64)
            adiag_y[i0 + 128 * t : i0 + 128 * t + 128] = np.diagonal(blk_y).astype(np.float64)

    def bracket(Pv, c1p, c2p, d1, d2_):
        n = float(N)
        r1 = c1p / (n - 2)
        r2 = c2p / (n - 2)
        t1 = c1p.sum() / ((n - 1) * (n - 2)) - K64 / (n - 1)
        t2 = c2p.sum() / ((n - 1) * (n - 2)) - K64 / (n - 1)
        sv = Pv
        sv += -2.0 * (r2 @ c1p) + t2 * c1p.sum()
        sv += -2.0 * (r1 @ c2p) + t1 * c2p.sum()
        sv += 4.0 * n * (r1 @ r2)
        sv += -2.0 * n * t2 * r1.sum() - 2.0 * n * t1 * r2.sum()
        sv += n * n * t1 * t2
        A_ii = (d1 - K64) - 2.0 * r1 + t1
        B_ii = (d2_ - K64) - 2.0 * r2 + t2
        sv -= (A_ii * B_ii).sum()
        return sv / (n * (n - 3.0))

    gxy = bracket(P[0], cspx, cspy, adiag_x, adiag_y)
    gxx = bracket(P[1], cspx, cspx, adiag_x, adiag_x)
    gyy = bracket(P[2], cspy, cspy, adiag_y, adiag_y)
    loss = -gxy / np.sqrt(gxx * gyy + EPS)
    return np.array(loss, dtype=np.float32)


def kernel(featuresX: np.ndarray, featuresY: np.ndarray) -> np.ndarray:
    nc = _get_nc()
    in_maps = _make_in_maps(featuresX, featuresY)
    _CACHE["in_maps"] = in_maps
    res = run_bass_kernel_spmd(nc, in_maps, list(range(NCORES))).results
    return _combine(res)


# revision 23
# speedup vs baseline: 1.1092x; 1.0051x over previous
"""Distance-correlation loss kernel for trn2 (8 NeuronCores, SPMD).

Reference math: for F in {X, Y}: a = sqrt(relu(sq_i + sq_j - 2 F F^T) + eps),
A = a - 2*row_j + tot (row = colsum/(n-2), tot = sum/((n-1)(n-2))), zero diag;
loss = -g_xy / sqrt(g_xx * g_yy + eps), g_PQ = sum(P*Q)/(n(n-3)).

Matrix-free single-pass formulation: with a' = a - 64, every bracket sum
expands as P' (= sum a'_x a'_y and squares) plus O(n) corrections from the
shifted colsums and the measured diagonal — the device computes, per tile,
only the distance tile, its shifted colsum, and three product partials. No
second pass, no collective; the host combines per-core partials in f64.

Symmetric schedule (a is symmetric -> compute ~half): 16 virtual half-blocks
of 256 rows, 2 per core. Core c streams 20 rotated j-chunks (global chunk
(4c+pos)%32); per position the tile is
  pos 0,1:   [128,256] left  (v0 self, weight 1)
  pos 2,3:   [128,512] left w2 + right v1-self w1
  pos 4..15: [128,512] both halves w2
  pos 16,17: [128,512] left w1 (d=8 pair, both orientations), right w2
  pos 18,19: [128,256] right w1 (odd d=8 pair)
Weight-2 halves get transposed-side column sums via ones^T @ a16 matmuls
accumulated in one shared PSUM bank (groups at partitions 0/32/64/96),
emitted with a lag so PE never waits on the ACT/GPSIMD chain. Weights are
baked into the DVE tensor_tensor_reduce `scale`.

Per tile: fp8(e4m3) DoubleRow matmuls (psum = x8_strip^T (-2 x8_core)) plus
one DoubleRow pair encoding sq_i - 2048; ACT sqrt with per-partition bias
sq_j + 2048 + 0.5 (+0.5 keeps the junk diagonal positive -> no relu/NaN);
GPSIMD tensor_scalar shift a-64 -> f16 with accum_out = shifted colsum; DVE
TTR partials. Diagonal blocks sit at stream positions 0..3; their a32 column
blocks are DMA'd out and the host subtracts the exact measured diagonal.

Pipelining: host pre-arranges every fp8 array in exact SBUF layout (fully
contiguous per partition -> no sub-512B DMA penalty); strips are half-width
(1.6us each) for fine overlap; x-strips ride SP, residents the scalar queue,
small tables + y-strips gpsimd; the x stream runs SKEW positions ahead of y
so the PE starts as soon as the first x strip lands.
"""

import sys

for _p in ("/opt/trn_rl_repo",):
    if _p not in sys.path:
        sys.path.insert(0, _p)

import numpy as np
import ml_dtypes

import concourse.bass as bass
from concourse import bacc
import concourse.mybir as mybir
import concourse.tile as tile
from concourse.bass_utils import run_bass_kernel_spmd

N = 4096
D = 2048
NCORES = 8
ROWS = N // NCORES          # 512 resident rows per core
NJ = N // 128               # 32 global j-chunks
NPOS = 20                   # streamed chunk positions per core
NKP = D // 256              # 8 DoubleRow contraction pairs
SKEW = 3                    # x stream runs this many positions ahead of y
K64 = 64.0
EB = 0.5
EPS = 1e-18
F32 = mybir.dt.float32
F16 = mybir.dt.float16
F8 = mybir.dt.float8e4
AF = mybir.ActivationFunctionType
ALU = mybir.AluOpType
DR = mybir.MatmulPerfMode.DoubleRow
f8 = ml_dtypes.float8_e4m3

_CACHE = {}


def _pos_tile(pos):
    """(tile_col_start, tile_width) within the core's 512 resident columns."""
    if pos < 2:
        return 0, 256
    if pos >= 18:
        return 256, 256
    return 0, 512


def _pos_ttrs(pos):
    """(col_start, width, weight) product segments for this position."""
    if pos < 2:
        return [(0, 256, 1.0)]
    if pos < 4:
        return [(0, 256, 2.0), (256, 256, 1.0)]
    if pos < 16:
        return [(0, 512, 2.0)]
    if pos < 18:
        return [(0, 256, 1.0), (256, 256, 2.0)]
    return [(256, 256, 1.0)]


def _pos_mirrors(pos):
    """Mirror groups fed at this position: list of (group, col_start)."""
    out = []
    if 2 <= pos <= 15:
        out.append((0, 0))
    if 4 <= pos <= 17:
        out.append((1, 256))
    return out


NACC = sum(len(_pos_ttrs(p)) for p in range(NPOS))   # accum columns per product
MIR_TOTAL = [14, 14]


def _build_nc():
    nc = bacc.Bacc(None, num_devices=NCORES, target_bir_lowering=False)

    # ---- inputs (pre-arranged in SBUF layout: [128, contiguous bytes]) ----
    # stream: 10 half-strips x [16 kchunks, 256 cols]
    xs8 = nc.declare_dram_parameter("xs8", [128, NPOS * 16 * 128], F8, isOutput=False)
    ys8 = nc.declare_dram_parameter("ys8", [128, NPOS * 16 * 128], F8, isOutput=False)
    # resident moving side (-2 x8): [16 kchunks, 512 cols]
    xm8 = nc.declare_dram_parameter("xm8", [128, 16 * ROWS], F8, isOutput=False)
    ym8 = nc.declare_dram_parameter("ym8", [128, 16 * ROWS], F8, isOutput=False)
    uabx = nc.declare_dram_parameter("uabx", [1, 2 * ROWS], F8, isOutput=False)
    uaby = nc.declare_dram_parameter("uaby", [1, 2 * ROWS], F8, isOutput=False)
    stat = nc.declare_dram_parameter("stat", [1, 256], F8, isOutput=False)
    onesf = nc.declare_dram_parameter("onesf", [128, 1], F16, isOutput=False)
    sqbx = nc.declare_dram_parameter("sqbx", [128, NPOS], F32, isOutput=False)
    sqby = nc.declare_dram_parameter("sqby", [128, NPOS], F32, isOutput=False)

    # ---- outputs ----
    csx = nc.declare_dram_parameter("csx", [128, NPOS], F32, isOutput=True)
    csy = nc.declare_dram_parameter("csy", [128, NPOS], F32, isOutput=True)
    accs = nc.declare_dram_parameter("accs", [128, 3 * NACC], F32, isOutput=True)
    adx = nc.declare_dram_parameter("adx", [128, 512], F32, isOutput=True)
    ady = nc.declare_dram_parameter("ady", [128, 512], F32, isOutput=True)
    mirs = nc.declare_dram_parameter("mirs", [128, 8], F32, isOutput=True)

    with tile.TileContext(nc) as tc:
        import contextlib

        with contextlib.ExitStack() as ctx:
            singles = ctx.enter_context(tc.tile_pool(name="singles", bufs=1))
            xstrips = ctx.enter_context(tc.tile_pool(name="xstrips", bufs=4))
            ystrips = ctx.enter_context(tc.tile_pool(name="ystrips", bufs=4))
            psum = ctx.enter_context(tc.tile_pool(name="psum", bufs=7, space="PSUM"))
            mpsum = ctx.enter_context(tc.tile_pool(name="mpsum", bufs=1, space="PSUM"))
            t32 = ctx.enter_context(tc.tile_pool(name="t32", bufs=6))
            t16 = ctx.enter_context(tc.tile_pool(name="t16", bufs=10))
            scrap = ctx.enter_context(tc.tile_pool(name="scrap", bufs=3))

            # ---- residents ----
            # big moving residents on the scalar (ACT) hwdge queue, split
            # into k-halves so the first matmuls start sooner
            xm_sb = singles.tile([128, 16, ROWS], F8, name="xm_sb")
            nc.scalar.dma_start(out=xm_sb[:, 0:8, :], in_=xm8[:, : 8 * ROWS])
            nc.scalar.dma_start(out=xm_sb[:, 8:16, :], in_=xm8[:, 8 * ROWS :])
            # ym rides SP between the first x strips (emitted later, below)
            ym_sb = singles.tile([128, 16, ROWS], F8, name="ym_sb")
            # small tables first on the gpsimd queue
            uabx_sb = singles.tile([1, 2, ROWS], F8, name="uabx_sb")
            nc.gpsimd.dma_start(out=uabx_sb[:], in_=uabx[:, :])
            stat_sb = singles.tile([1, 2, 128], F8, name="stat_sb")
            nc.gpsimd.dma_start(out=stat_sb[:], in_=stat[:, :])
            sqbx_sb = singles.tile([128, NPOS], F32, name="sqbx_sb")
            nc.gpsimd.dma_start(out=sqbx_sb[:], in_=sqbx[:, :])
            uaby_sb = singles.tile([1, 2, ROWS], F8, name="uaby_sb")
            nc.gpsimd.dma_start(out=uaby_sb[:], in_=uaby[:, :])
            sqby_sb = singles.tile([128, NPOS], F32, name="sqby_sb")
            nc.gpsimd.dma_start(out=sqby_sb[:], in_=sqby[:, :])
            ones_sb = singles.tile([128, 1], F16, name="ones_sb")
            nc.gpsimd.dma_start(out=ones_sb[:], in_=onesf[:, :])

            csx_sb = singles.tile([128, NPOS], F32, name="csx_sb")
            csy_sb = singles.tile([128, NPOS], F32, name="csy_sb")
            accs_sb = singles.tile([128, 3 * NACC], F32, name="accs_sb")
            mir_sb = singles.tile([128, 8], F32, name="mir_sb")

            # Mirror sums via the stationary trick: lhsT = a16 column chunk,
            # rhs = ones [128,1] -> out[c,0] = sum_p a16[p,c]. Output free
            # size 1 makes these matmuls ~free on the PE. All 8 accumulator
            # columns (m,g,chunk) share one PSUM bank: the first matmul's
            # start=True zeroes the whole 2KB region (emission order on the
            # in-order PE guarantees it runs first), the very last carries
            # stop=True.
            mir_ps = mpsum.tile([128, 8], F32, name="mir_ps")
            mir_emitted = [0]
            MIR_MM_TOTAL = 2 * 2 * MIR_TOTAL[0] * 2   # m * chunks * positions
            pending_mirrors = []

            def flush_mirror():
                m, g, a16t, rel = pending_mirrors.pop(0)
                for chunk in range(2):
                    col = 4 * m + 2 * g + chunk
                    mir_emitted[0] += 1
                    nc.tensor.matmul(
                        mir_ps[:, col : col + 1],
                        lhsT=a16t[:, rel + 128 * chunk : rel + 128 * chunk + 128],
                        rhs=ones_sb[:],
                        start=(mir_emitted[0] == 1),
                        stop=(mir_emitted[0] == MIR_MM_TOTAL),
                    )
                if mir_emitted[0] == MIR_MM_TOTAL:
                    nc.scalar.activation(
                        mir_sb[:], mir_ps[:],
                        AF.Copy, bias=0.0, scale=1.0,
                    )

            strips = [[None] * (NPOS // 2) for _ in range(2)]

            def load_strip(m, h):
                pool, eng = (xstrips, nc.sync) if m == 0 else (ystrips, nc.gpsimd)
                src = xs8 if m == 0 else ys8
                st = pool.tile([128, 16, 256], F8, tag="st")
                eng.dma_start(out=st[:], in_=src[:, 4096 * h : 4096 * (h + 1)])
                strips[m][h] = st

            sides = (
                (xm_sb, uabx_sb, sqbx_sb, csx_sb, adx),
                (ym_sb, uaby_sb, sqby_sb, csy_sb, ady),
            )
            a16_live = [{}, {}]
            acc_col_of = {}
            _c = 0
            for pos in range(NPOS):
                acc_col_of[pos] = _c
                _c += len(_pos_ttrs(pos))

            def emit_tile(m, pos):
                m_sb, uab_sb, sqb_sb, cs_sb, ad = sides[m]
                h = pos // 2
                t = pos % 2
                if strips[m][h] is None:
                    load_strip(m, h)
                    if h + 1 < NPOS // 2 and strips[m][h + 1] is None:
                        load_strip(m, h + 1)
                strip = strips[m][h]
                c0, cw = _pos_tile(pos)
                ps = psum.tile([128, cw], F32, tag="mm")
                for kp in range(NKP):
                    nc.tensor.matmul(
                        ps[:],
                        lhsT=strip[:, 2 * kp : 2 * kp + 2, 128 * t : 128 * t + 128],
                        rhs=m_sb[:, 2 * kp : 2 * kp + 2, c0 : c0 + cw],
                        start=(kp == 0),
                        stop=False,
                        perf_mode=DR,
                    )
                nc.tensor.matmul(
                    ps[:], lhsT=stat_sb[:], rhs=uab_sb[:, :, c0 : c0 + cw],
                    start=False, stop=True, perf_mode=DR,
                )
                a32 = t32.tile([128, cw], F32, tag="a32")
                nc.scalar.activation(
                    a32[:], ps[:], AF.Sqrt,
                    bias=sqb_sb[:, pos : pos + 1], scale=1.0,
                )
                a16 = t16.tile([128, cw], F16, tag="a16")
                nc.gpsimd.tensor_scalar(
                    a16[:], a32[:], -K64, None,
                    op0=ALU.add, op1=ALU.add,
                    accum_out=cs_sb[:, pos : pos + 1],
                )
                a16_live[m][pos] = a16
                if pos < 4:
                    nc.sync.dma_start(
                        out=ad[:, 128 * pos : 128 * pos + 128],
                        in_=a32[:, 128 * pos - c0 : 128 * pos - c0 + 128],
                    )
                for g, gcol in _pos_mirrors(pos):
                    pending_mirrors.append((m, g, a16, gcol - c0))

            # bb products of these positions run on ACT as Square(sqrt(2)*x)
            # (weight 2 baked into scale^2) to unload DVE
            ACT_BB = set(range(6, 15))

            def emit_ttrs(pos):
                c0, _ = _pos_tile(pos)
                a16x = a16_live[0].pop(pos)
                a16y = a16_live[1][pos]
                acc_col = acc_col_of[pos]
                for seg0, segw, segwt in _pos_ttrs(pos):
                    for k, (i0t, i1t) in enumerate(
                        ((a16x, a16y), (a16x, a16x), (a16y, a16y))
                    ):
                        acc_ap = accs_sb[:, k * NACC + acc_col : k * NACC + acc_col + 1]
                        if k == 2 and pos in ACT_BB:
                            scrb = scrap.tile([128, segw], F16, tag="scr")
                            nc.scalar.activation(
                                scrb[:],
                                i1t[:, seg0 - c0 : seg0 - c0 + segw],
                                AF.Square, bias=0.0, scale=float(np.sqrt(segwt)),
                                accum_out=acc_ap,
                            )
                            continue
                        scr = scrap.tile([128, segw], F16, tag="scr")
                        nc.vector.tensor_tensor_reduce(
                            out=scr[:],
                            in0=i0t[:, seg0 - c0 : seg0 - c0 + segw],
                            in1=i1t[:, seg0 - c0 : seg0 - c0 + segw],
                            scale=segwt, scalar=0.0,
                            op0=ALU.mult, op1=ALU.add,
                            accum_out=acc_ap,
                        )
                    acc_col += 1
                a16_live[1].pop(pos)

            # heavy [512] positions first; light [256] self/d8 tiles last so
            # the drain chain is short
            ORDER = list(range(2, 18)) + [0, 1, 18, 19]
            # prime the first two distinct x strips
            primed = []
            for o in ORDER:
                if o // 2 not in primed:
                    primed.append(o // 2)
                if len(primed) == 2:
                    break
            for h in primed:
                load_strip(0, h)
            nc.sync.dma_start(out=ym_sb[:, 0:8, :], in_=ym8[:, : 8 * ROWS])
            nc.sync.dma_start(out=ym_sb[:, 8:16, :], in_=ym8[:, 8 * ROWS :])
            # y starts SKEW positions behind x (residents arrive later) and
            # catches up to 1 behind mid-stream so the drain tail is short
            y_next = 0
            for i in range(NPOS + 1):
                ycount = 0 if i < SKEW else (2 if i in (NPOS // 2, NPOS // 2 + 1) else 1)
                if i == NPOS:
                    ycount = NPOS - y_next
                for _ in range(ycount):
                    if y_next < NPOS and y_next < i:
                        pos = ORDER[y_next]
                        emit_tile(1, pos)
                        emit_ttrs(pos)
                        y_next += 1
                if i < NPOS:
                    emit_tile(0, ORDER[i])
                while len(pending_mirrors) > 6:
                    flush_mirror()
            while y_next < NPOS:
                pos = ORDER[y_next]
                emit_tile(1, pos)
                emit_ttrs(pos)
                y_next += 1
            while pending_mirrors:
                flush_mirror()

            nc.sync.dma_start(out=csx[:, :], in_=csx_sb[:])
            nc.sync.dma_start(out=csy[:, :], in_=csy_sb[:])
            nc.sync.dma_start(out=accs[:, :], in_=accs_sb[:])
            nc.sync.dma_start(out=mirs[:, :], in_=mir_sb[:])

    nc.compile()
    return nc


def _get_nc():
    if "nc" not in _CACHE:
        _CACHE["nc"] = _build_nc()
    return _CACHE["nc"]


def _prep_side(F):
    x8 = np.asarray(F, dtype=np.float32).reshape(N, D).astype(f8).astype(np.float32)
    xsT = np.ascontiguousarray(x8.T).astype(f8)                 # [D, N]
    xmT = np.ascontiguousarray((-2.0 * x8).T).astype(f8)        # [D, N]
    sq = np.einsum("ij,ij->i", x8.astype(np.float64), x8.astype(np.float64))
    u = sq - 2048.0
    uA = (u / 16.0).astype(f8)
    uB = ((u - uA.astype(np.float64) * 16.0) / 2.0).astype(f8)
    sqb = (sq + 2048.0 + EB).astype(np.float32)
    return xsT, xmT, np.asarray(uA), np.asarray(uB), sqb


def _sbuf_arrange_stream(arr, c):
    """[D, N] -> [128, NPOS*16*128]: half-strip h holds k-chunks of rotated
    columns [128h, 128h+128) x [128 cols] in [k][col] order per partition."""
    start = 512 * c
    end = start + NPOS * 128
    if end <= N:
        w = arr[:, start:end]
    else:
        w = np.concatenate([arr[:, start:], arr[:, : end - N]], axis=1)
    # w: [D, NPOS*128]; per partition p: [halfstrip][k][col256], D = (k p)
    v = w.reshape(16, 128, NPOS // 2, 256)       # [k, p, hs, col]
    v = v.transpose(1, 2, 0, 3)                  # [p, hs, k, col]
    return np.ascontiguousarray(v.reshape(128, NPOS * 16 * 128))


def _sbuf_arrange_resident(arr_sl):
    """[D, ROWS] -> [128, 16*ROWS] in [k][col] order per partition."""
    v = arr_sl.reshape(16, 128, ROWS)            # [k, p, col]
    v = v.transpose(1, 0, 2)                     # [p, k, col]
    return np.ascontiguousarray(v.reshape(128, 16 * ROWS))


def _make_in_maps(featuresX, featuresY):
    xsT, xmT, uAx, uBx, sqbx = _prep_side(featuresX)
    ysT, ymT, uAy, uBy, sqby = _prep_side(featuresY)
    stat_np = np.concatenate(
        [np.full(128, 16.0, np.float32), np.full(128, 2.0, np.float32)]
    ).astype(f8).reshape(1, 256)
    ones_np = np.ones((128, 1), np.float16)

    in_maps = []
    for c in range(NCORES):
        sl = slice(c * ROWS, (c + 1) * ROWS)
        rot = [(4 * c + pos) % NJ for pos in range(NPOS)]
        sqbx_c = np.stack([sqbx[128 * g : 128 * g + 128] for g in rot], axis=1)
        sqby_c = np.stack([sqby[128 * g : 128 * g + 128] for g in rot], axis=1)
        in_maps.append(
            {
                "xs8": _sbuf_arrange_stream(xsT, c),
                "ys8": _sbuf_arrange_stream(ysT, c),
                "xm8": _sbuf_arrange_resident(xmT[:, sl]),
                "ym8": _sbuf_arrange_resident(ymT[:, sl]),
                "uabx": np.concatenate([uAx[sl], uBx[sl]]).reshape(1, 2 * ROWS),
                "uaby": np.concatenate([uAy[sl], uBy[sl]]).reshape(1, 2 * ROWS),
                "stat": stat_np,
                "onesf": ones_np,
                "sqbx": np.ascontiguousarray(sqbx_c),
                "sqby": np.ascontiguousarray(sqby_c),
            }
        )
    return in_maps


def _combine(res):
    cspx = np.zeros(N, np.float64)
    cspy = np.zeros(N, np.float64)
    P = np.zeros(3, np.float64)
    adiag_x = np.zeros(N, np.float64)
    adiag_y = np.zeros(N, np.float64)
    for c in range(NCORES):
        r = res[c]
        for pos in range(NPOS):
            gj = (4 * c + pos) % NJ
            cspx[128 * gj : 128 * gj + 128] += r["csx"][:, pos].astype(np.float64)
            cspy[128 * gj : 128 * gj + 128] += r["csy"][:, pos].astype(np.float64)
        P += r["accs"].astype(np.float64).reshape(128, 3, NACC).sum(axis=(0, 2))
        i0 = 512 * c
        mir = r["mirs"].astype(np.float64)   # [p, 4m+2g+chunk]
        for m, csp in ((0, cspx), (1, cspy)):
            for g in range(2):
                for chunk in range(2):
                    b = i0 + 256 * g + 128 * chunk
                    csp[b : b + 128] += mir[:, 4 * m + 2 * g + chunk]
        for t in range(4):
            blk_x = r["adx"][:, 128 * t : 128 * t + 128]
            blk_y = r["ady"][:, 128 * t : 128 * t + 128]
            adiag_x[i0 + 128 * t : i0 + 128 * t + 128] = np.diagonal(blk_x).astype(np.float64)
            adiag_y[i0 + 128 * t : i0 + 128 * t + 128] = np.diagonal(blk_y).astype(np.float64)

    def bracket(Pv, c1p, c2p, d1, d2_):
        n = float(N)
        r1 = c1p / (n - 2)
        r2 = c2p / (n - 2)
        t1 = c1p.sum() / ((n - 1) * (n - 2)) - K64 / (n - 1)
        t2 = c2p.sum() / ((n - 1) * (n - 2)) - K64 / (n - 1)
        sv = Pv
        sv += -2.0 * (r2 @ c1p) + t2 * c1p.sum()
        sv += -2.0 * (r1 @ c2p) + t1 * c2p.sum()
        sv += 4.0 * n * (r1 @ r2)
        sv += -2.0 * n * t2 * r1.sum() - 2.0 * n * t1 * r2.sum()
        sv += n * n * t1 * t2
        A_ii = (d1 - K64) - 2.0 * r1 + t1
        B_ii = (d2_ - K64) - 2.0 * r2 + t2
        sv -= (A_ii * B_ii).sum()
        return sv / (n * (n - 3.0))

    gxy = bracket(P[0], cspx, cspy, adiag_x, adiag_y)
    gxx = bracket(P[1], cspx, cspx, adiag_x, adiag_x)
    gyy = bracket(P[2], cspy, cspy, adiag_y, adiag_y)
    loss = -gxy / np.sqrt(gxx * gyy + EPS)
    return np.array(loss, dtype=np.float32)


def kernel(featuresX: np.ndarray, featuresY: np.ndarray) -> np.ndarray:
    nc = _get_nc()
    in_maps = _make_in_maps(featuresX, featuresY)
    _CACHE["in_maps"] = in_maps
    res = run_bass_kernel_spmd(nc, in_maps, list(range(NCORES))).results
    return _combine(res)
